# revision 9
# baseline (speedup 1.0000x reference)
"""Distributed Trainium2 (8 NeuronCores) kernel for a 2-layer GATv2 GNN.

Self-contained: takes full inputs, shards by destination node across 8 cores,
runs one SPMD Bass kernel (compiled at call time, cached per process), and
returns the full (50000, 128) output.

Sharding: nodes are relabeled so that in-degrees are balanced across 392
dst-blocks of 128 nodes (greedy LPT); each core owns 49 blocks. Edges live on
the core owning their destination; segment softmax and scatter-add are
device-local. Source features cross cores via an AllGather of the per-layer
source projection (bf16), then per-edge rows are fetched with dma_gather from
the gathered table (2 nodes packed per 512B element so indices fit in int16).
"""

import os, sys, math
sys.path.insert(0, "/opt/trn_rl_repo")
os.environ.setdefault("JAX_PLATFORMS", "")

import numpy as np
import ml_dtypes

import concourse.bacc as bacc
import concourse.bass as bass
import concourse.mybir as mybir
import concourse.tile as tile
from concourse.bass_utils import run_bass_kernel_spmd

F32 = mybir.dt.float32
BF16 = mybir.dt.bfloat16
I16 = mybir.dt.int16
U8 = mybir.dt.uint8
Alu = mybir.AluOpType
Act = mybir.ActivationFunctionType
AxX = mybir.AxisListType.X

NCORES = 8
BLK = 128
NEG_SLOPE = 0.2
LN_EPS = 1e-5
H, C = 4, 32
HID = H * C  # 128


# ---------------------------------------------------------------- host prep

def _balance_blocks(dst, n_nodes, n_blocks):
    """Greedy LPT: assign nodes to n_blocks blocks of <=128 nodes balancing
    in-degree sums. Returns block_of_node (int32)."""
    import heapq
    deg = np.bincount(dst, minlength=n_nodes).astype(np.int64)
    order = np.argsort(-deg, kind="stable")
    heap = [(0, b) for b in range(n_blocks)]
    heapq.heapify(heap)
    counts = np.zeros(n_blocks, np.int32)
    block_of = np.empty(n_nodes, np.int32)
    stash = []
    for nid in order:
        d = int(deg[nid])
        while True:
            s, b = heapq.heappop(heap)
            if counts[b] < BLK:
                break
            # full block: drop it permanently
        block_of[nid] = b
        counts[b] += 1
        if counts[b] < BLK:
            heapq.heappush(heap, (s + d, b))
        else:
            stash.append((s + d, b))
    return block_of, deg


def _wrap_idx(idx1d):
    """int16 (E,) -> (128, E//16) wrapped-16 + replicated layout."""
    e = idx1d.shape[0]
    assert e % 16 == 0
    w = idx1d.reshape(e // 16, 16).T  # (16, S)
    return np.tile(w, (8, 1)).copy()


def _prep(x, edge_attr, edge_index, nblk_per_core=49):
    n, din = x.shape
    e = edge_index.shape[1]
    n_blocks = NCORES * nblk_per_core  # 392
    npad = n_blocks * BLK              # 50176
    npc = nblk_per_core * BLK          # 6272

    src, dst = edge_index[0].astype(np.int64), edge_index[1].astype(np.int64)
    block_of, deg = _balance_blocks(dst, n, n_blocks)

    # new node ids: block b slots
    order = np.argsort(block_of, kind="stable")
    new_id = np.empty(npad, np.int64)
    # slot within block
    counts = np.bincount(block_of, minlength=n_blocks)
    starts = np.zeros(n_blocks + 1, np.int64)
    np.cumsum(counts, out=starts[1:])
    slot = np.empty(n, np.int64)
    slot[order] = np.arange(n) - starts[block_of[order]]
    new_of_old = block_of.astype(np.int64) * BLK + slot  # old -> new
    # per-block edge counts and capacity
    eb = np.bincount(block_of[dst], minlength=n_blocks)
    cb = int(math.ceil(eb.max() / BLK))
    ecap = cb * BLK

    new_src = new_of_old[src]
    new_dst = new_of_old[dst]
    dblk = new_dst // BLK

    # order edges by destination block
    eorder = np.argsort(dblk, kind="stable")
    s_src = new_src[eorder]
    s_dst = new_dst[eorder]
    s_att = edge_attr[eorder]
    s_blk = dblk[eorder]

    # slot edges into (n_blocks, ecap) with padding
    eoff = np.zeros(n_blocks + 1, np.int64)
    np.cumsum(eb, out=eoff[1:])
    in_blk_pos = np.arange(e) - eoff[s_blk]
    flat_pos = s_blk * ecap + in_blk_pos

    g_srcpk = np.zeros(n_blocks * ecap, np.int16)
    g_par = np.zeros(n_blocks * ecap, np.uint8)
    g_drel = np.full(n_blocks * ecap, -1.0, np.float32)
    g_attT = np.zeros((n_blocks * ecap, edge_attr.shape[1]), np.float32)
    g_srcpk[flat_pos] = (s_src // 2).astype(np.int16)
    g_par[flat_pos] = (s_src % 2).astype(np.uint8)
    g_drel[flat_pos] = (s_dst % BLK).astype(np.float32)
    g_attT[flat_pos] = s_att

    g_srcpk = g_srcpk.reshape(n_blocks, ecap)
    g_par = g_par.reshape(n_blocks, ecap)
    g_drel = g_drel.reshape(n_blocks, ecap)
    g_attT = g_attT.reshape(n_blocks, ecap, -1)

    # x in new order, padded, transposed per core
    x_new = np.zeros((npad, din), np.float32)
    x_new[new_of_old] = x

    per_core = []
    for c in range(NCORES):
        b0, b1 = c * nblk_per_core, (c + 1) * nblk_per_core
        srcpk = g_srcpk[b0:b1]
        # wrapped idx layout per block: (49, 128, cb*8)
        eidx = np.stack([_wrap_idx(srcpk[i]) for i in range(nblk_per_core)])
        # (49, 128, cb) partition-major per chunk: elem j of chunk k sits at
        # partition j, free k  -> arr[b, p, k] = val[b, k*128 + p]
        def pmaj(a, dt):
            return np.ascontiguousarray(
                a.reshape(nblk_per_core, cb, BLK).transpose(0, 2, 1)).astype(dt)
        epar = pmaj(g_par[b0:b1], np.uint8)
        edrel = pmaj(g_drel[b0:b1], np.float32)
        # replicated dst_rel for onehotT: (49, 128, ecap) uint8 (pad -> 255)
        dr = g_drel[b0:b1].copy()
        dr[dr < 0] = 255.0
        edrr = np.broadcast_to(dr[:, None, :].astype(np.uint8),
                               (nblk_per_core, 128, ecap)).copy()
        eaT = np.ascontiguousarray(
            g_attT[b0:b1].transpose(0, 2, 1)).astype(np.float32)  # (49,16,ecap)
        xT = np.ascontiguousarray(x_new[c * npc:(c + 1) * npc].T)  # (din, 6272)
        per_core.append(dict(eidx=eidx, epar=epar, edrel=edrel, edrr=edrr,
                             eaT=eaT, xT=xT))
    meta = dict(cb=cb, npad=npad, npc=npc, nblk=nblk_per_core,
                new_of_old=new_of_old, din=din, ed=edge_attr.shape[1])
    return per_core, meta


# ---------------------------------------------------------------- builder

def _rep(v, rows=128):
    v = np.asarray(v, np.float32).ravel()
    return np.broadcast_to(v, (rows, v.shape[0])).copy()


def _bf(a):
    return np.asarray(a).astype(ml_dtypes.bfloat16)


def build_graph(cb, din, ed, nblk=49, debug=False):
    npc = nblk * BLK
    ecap = cb * BLK
    npad = NCORES * npc
    nc = bacc.Bacc(None, target_bir_lowering=False)

    dp = nc.declare_dram_parameter
    # per-core data
    xT_a = dp("xT_a", [128, nblk, 128], BF16, isOutput=False)
    xT_b = dp("xT_b", [128, nblk, 128], BF16, isOutput=False)
    xT_c = dp("xT_c", [din - 256, nblk, 128], BF16, isOutput=False)
    eidx = dp("eidx", [nblk, 128, ecap // 16], I16, isOutput=False)
    epar = dp("epar", [nblk, 128, cb], U8, isOutput=False)
    edrel = dp("edrel", [nblk, 128, cb], F32, isOutput=False)
    edrr = dp("edrr", [nblk, 128, ecap], U8, isOutput=False)
    eaT = dp("eaT", [nblk, ed, ecap], BF16, isOutput=False)
    # replicated weights
    w1 = dp("w1", [din, 384], BF16, isOutput=False)      # [Wl1|Wr1|Wres]
    w2 = dp("w2", [HID, 256], BF16, isOutput=False)      # [Wl2|Wr2]
    wo = dp("wo", [HID, 128], BF16, isOutput=False)
    we1 = dp("we1", [ed, 128], BF16, isOutput=False)
    we2 = dp("we2", [ed, 128], BF16, isOutput=False)
    b1r = dp("b1r", [128, 384], F32, isOutput=False)     # [bl1|br1|bres] rep
    b2r = dp("b2r", [128, 256], F32, isOutput=False)
    bias1r = dp("bias1r", [128, 128], F32, isOutput=False)
    bias2r = dp("bias2r", [128, 128], F32, isOutput=False)
    bor = dp("bor", [128, 128], F32, isOutput=False)
    att1r = dp("att1r", [128, 128], BF16, isOutput=False)
    att2r = dp("att2r", [128, 128], BF16, isOutput=False)
    g1r = dp("g1r", [128, 128], F32, isOutput=False)
    bn1r = dp("bn1r", [128, 128], F32, isOutput=False)
    g2r = dp("g2r", [128, 128], F32, isOutput=False)
    bn2r = dp("bn2r", [128, 128], F32, isOutput=False)
    gor = dp("gor", [128, 128], F32, isOutput=False)
    bo2r = dp("bo2r", [128, 128], F32, isOutput=False)
    iotar = dp("iotar", [128, 128], F32, isOutput=False)
    iotac = dp("iotac", [128, 1], F32, isOutput=False)
    identm = dp("identm", [128, 128], F32, isOutput=False)
    out_ext = dp("out", [npc, 128], F32, isOutput=True)

    xl1d = nc.dram_tensor("xl1d", [npc, 128], BF16)
    if debug:
        dbg_hraw = nc.declare_dram_parameter("dbg_hraw", [npc, 128], F32, isOutput=True)
        dbg_h1 = nc.declare_dram_parameter("dbg_h1", [npc, 128], F32, isOutput=True)
        dbg_xls = nc.declare_dram_parameter("dbg_xls", [128, cb, 128], F32, isOutput=True)
        dbg_sc = nc.declare_dram_parameter("dbg_sc", [128, cb, 4], F32, isOutput=True)
    xl2d = nc.dram_tensor("xl2d", [npc, 128], BF16)
    ag1 = nc.dram_tensor("ag1", [npad, 128], BF16, addr_space="Shared")
    ag2 = nc.dram_tensor("ag2", [npad, 128], BF16, addr_space="Shared")

    with tile.TileContext(nc) as tc:
        with (
            tc.tile_pool(name="const", bufs=1) as cpool,
            tc.tile_pool(name="bigb", bufs=3) as bigb,
            tc.tile_pool(name="bigf", bufs=3) as bigf,
            tc.tile_pool(name="bigc", bufs=1) as bigc,
            tc.tile_pool(name="work", bufs=2) as wp,
            tc.tile_pool(name="small", bufs=3) as sp,
            tc.tile_pool(name="psA", bufs=2, space="PSUM") as psA,
            tc.tile_pool(name="psB", bufs=2, space="PSUM") as psB,
            tc.tile_pool(name="psC", bufs=2, space="PSUM") as psC,
        ):
            # ---- constants in SBUF
            def cload(ap, shape, dt, tag):
                t = cpool.tile(shape, dt, tag=tag)
                if hasattr(ap, 'shape') and not isinstance(ap, bass.AP):
                    ap = ap[:]
                nc.sync.dma_start(out=t[:], in_=ap)
                return t

            w1a = cload(w1[0:128, :], [128, 384], BF16, "w1a")
            w1b = cload(w1[128:256, :], [128, 384], BF16, "w1b")
            w1c = cload(w1[256:din, :], [din - 256, 384], BF16, "w1c")
            w2s = cload(w2[:], [HID, 256], BF16, "w2s")
            wos = cload(wo[:], [HID, 128], BF16, "wos")
            we1s = cload(we1[:], [ed, 128], BF16, "we1s")
            we2s = cload(we2[:], [ed, 128], BF16, "we2s")
            b1s = cload(b1r[:], [128, 384], F32, "b1s")
            b2s = cload(b2r[:], [128, 256], F32, "b2s")
            bias1s = cload(bias1r[:], [128, 128], F32, "bias1s")
            bias2s = cload(bias2r[:], [128, 128], F32, "bias2s")
            bos = cload(bor[:], [128, 128], F32, "bos")
            att1s = cload(att1r[:], [128, 128], BF16, "att1s")
            att2s = cload(att2r[:], [128, 128], BF16, "att2s")
            g1s = cload(g1r[:], [128, 128], F32, "g1s")
            bn1s = cload(bn1r[:], [128, 128], F32, "bn1s")
            g2s = cload(g2r[:], [128, 128], F32, "g2s")
            bn2s = cload(bn2r[:], [128, 128], F32, "bn2s")
            gos = cload(gor[:], [128, 128], F32, "gos")
            bo2s = cload(bo2r[:], [128, 128], F32, "bo2s")
            iotas = cload(iotar[:], [128, 128], F32, "iotas")
            iotacs = cload(iotac[:], [128, 1], F32, "iotacs")
            idents = cload(identm[:], [128, 128], F32, "idents")

            # ---- persistent big tiles
            xa = bigb.tile([128, nblk, 128], BF16, tag="bb")
            xb = bigb.tile([128, nblk, 128], BF16, tag="bb")
            xc_ = bigc.tile([din - 256, nblk, 128], BF16, tag="bbc")
            nc.sync.dma_start(out=xa[:], in_=xT_a[:])
            nc.sync.dma_start(out=xb[:], in_=xT_b[:])
            nc.sync.dma_start(out=xc_[:], in_=xT_c[:])

            res1 = bigf.tile([128, nblk, 128], F32, tag="bf")
            xr1 = bigb.tile([128, nblk, 128], BF16, tag="bb")

            # ---- P1: layer-1 projections per node tile
            for t in range(nblk):
                ps = psA.tile([128, 384], F32, tag="pa")
                nc.tensor.matmul(ps[:], xa[:, t, :], w1a[:], start=True, stop=False)
                nc.tensor.matmul(ps[:], xb[:, t, :], w1b[:], start=False, stop=False)
                nc.tensor.matmul(ps[:], xc_[:, t, :], w1c[:], start=False, stop=True)
                pb = wp.tile([128, 384], F32, tag="projf")
                nc.vector.tensor_tensor(out=pb[:], in0=ps[:], in1=b1s[:], op=Alu.add)
                xlb = wp.tile([128, 128], BF16, tag="projb")
                nc.vector.tensor_copy(xlb[:], pb[:, 0:128])
                nc.sync.dma_start(out=xl1d[t * 128:(t + 1) * 128, :], in_=xlb[:])
                nc.vector.tensor_copy(xr1[:, t, :], pb[:, 128:256])
                nc.vector.tensor_copy(res1[:, t, :], pb[:, 256:384])

            nc.gpsimd.collective_compute(
                "AllGather", Alu.bypass,
                replica_groups=[list(range(NCORES))],
                ins=[xl1d[:]], outs=[ag1[:]],
            )

            # ---- edge stage (shared for both layers)
            def edge_layer(ag, xr, att_s, we_s, bias_s, res, gamma_s, beta_s,
                           hout, hout_tap=False):
                agp = ag[:].rearrange("(n two) f -> n (two f)", two=2)
                for b in range(nblk):
                    idxt = wp.tile([128, ecap // 16], I16, tag="idxt")
                    nc.sync.dma_start(out=idxt[:], in_=eidx[b])
                    part = wp.tile([128, cb], U8, tag="part")
                    nc.sync.dma_start(out=part[:], in_=epar[b])
                    drel = wp.tile([128, cb], F32, tag="drel")
                    nc.sync.dma_start(out=drel[:], in_=edrel[b])
                    drr = wp.tile([128, ecap], U8, tag="drr")
                    nc.sync.dma_start(out=drr[:], in_=edrr[b])
                    eat = wp.tile([ed, ecap], BF16, tag="eat")
                    nc.sync.dma_start(out=eat[:], in_=eaT[b])

                    gth = wp.tile([128, cb, 256], BF16, tag="gth")
                    nc.gpsimd.dma_gather(
                        out_ap=gth[:], in_ap=agp, idxs_ap=idxt[:],
                        num_idxs=ecap, num_idxs_reg=ecap, elem_size=256,
                        single_packet=False)

                    xls = wp.tile([128, cb, 128], BF16, tag="xls")
                    nc.vector.tensor_copy(xls[:], gth[:, :, 0:128])
                    for k in range(cb):
                        nc.vector.copy_predicated(
                            xls[:, k, :],
                            part[:, k:k + 1].broadcast_to([128, 128]),
                            gth[:, k, 128:256])

                    oh = wp.tile([128, cb, 128], BF16, tag="oh")
                    nc.vector.tensor_tensor(
                        out=oh[:],
                        in0=iotas[:].unsqueeze(1).broadcast_to([128, cb, 128]),
                        in1=drel[:].unsqueeze(2).broadcast_to([128, cb, 128]),
                        op=Alu.is_equal)
                    ohT = wp.tile([128, ecap], BF16, tag="ohT")
                    nc.vector.tensor_scalar(
                        out=ohT[:], in0=drr[:], scalar1=iotacs[:], scalar2=None,
                        op0=Alu.is_equal)

                    # q = onehotT^T @ xr_blk + eaT^T @ We ; m = xls + q
                    mt = wp.tile([128, cb, 128], BF16, tag="mt")
                    for k in range(cb):
                        qps = psB.tile([128, 128], F32, tag="pb")
                        nc.tensor.matmul(qps[:], ohT[:, k * 128:(k + 1) * 128],
                                         xr[:, b, :], start=True, stop=False)
                        nc.tensor.matmul(qps[:], eat[:, k * 128:(k + 1) * 128],
                                         we_s[:], start=False, stop=True)
                        nc.vector.tensor_tensor(out=mt[:, k, :], in0=xls[:, k, :],
                                                in1=qps[:], op=Alu.add)

                    if debug and hout_tap and b == 0:
                        xcp = wp.tile([128, cb, 128], F32, tag="dbgx")
                        nc.vector.tensor_copy(xcp[:], mt[:])
                        nc.sync.dma_start(out=dbg_xls[:], in_=xcp[:])
                    # leaky relu: g = max(m, 0.2 m)
                    m2 = wp.tile([128, cb, 128], BF16, tag="m2")
                    nc.scalar.mul(m2[:], mt[:], NEG_SLOPE)
                    nc.vector.tensor_tensor(out=mt[:], in0=mt[:], in1=m2[:],
                                            op=Alu.max)
                    # scores: reuse m2 as g*att
                    gs = m2
                    nc.vector.tensor_tensor(
                        out=gs[:], in0=mt[:],
                        in1=att_s[:].unsqueeze(1).broadcast_to([128, cb, 128]),
                        op=Alu.mult)
                    sc = sp.tile([128, cb, H], F32, tag="sc")
                    nc.vector.tensor_reduce(
                        out=sc[:], in_=gs[:].rearrange("p c (h j) -> p c h j", h=H),
                        op=Alu.add, axis=AxX)
                    if debug and hout_tap and b == 0:
                        nc.sync.dma_start(out=dbg_sc[:], in_=sc[:])
                    ex = sp.tile([128, cb, H], F32, tag="ex")
                    nc.scalar.activation(ex[:], sc[:], Act.Exp)

                    rhsb = wp.tile([128, cb, 132], BF16, tag="rhsb")
                    nc.vector.tensor_tensor(
                        out=rhsb[:, :, 0:128].rearrange("p c (h j) -> p c h j", h=H),
                        in0=xls[:].rearrange("p c (h j) -> p c h j", h=H),
                        in1=ex[:].unsqueeze(3).broadcast_to([128, cb, H, C]),
                        op=Alu.mult)
                    nc.vector.tensor_copy(rhsb[:, :, 128:132], ex[:])

                    scps = psC.tile([128, 132], F32, tag="pc")
                    for k in range(cb):
                        nc.tensor.matmul(scps[:], oh[:, k, :], rhsb[:, k, :],
                                         start=(k == 0), stop=(k == cb - 1))

                    st = sp.tile([128, H], F32, tag="st")
                    nc.vector.tensor_scalar(out=st[:], in0=scps[:, 128:132],
                                            scalar1=1e-16, scalar2=None,
                                            op0=Alu.add)
                    srec = sp.tile([128, H], F32, tag="srec")
                    nc.vector.reciprocal(srec[:], st[:])
                    nc.vector.tensor_tensor(
                        out=hout[:, b, :].rearrange("p (h j) -> p h j", h=H),
                        in0=scps[:, 0:128].rearrange("p (h j) -> p h j", h=H),
                        in1=srec[:].unsqueeze(2).broadcast_to([128, H, C]),
                        op=Alu.mult)

                if debug and hout_tap:
                    for bb in range(nblk):
                        tcp = wp.tile([128, 128], F32, tag="dbgcp")
                        nc.vector.tensor_copy(tcp[:], hout[:, bb, :])
                        nc.sync.dma_start(out=dbg_hraw[bb * 128:(bb + 1) * 128, :], in_=tcp[:])
                # batched: h = LN(elu(h + bias) + res)
                s1 = bigf.tile([128, nblk, 128], F32, tag="bf")
                hf = hout[:]
                nc.vector.tensor_tensor(
                    out=hf, in0=hf,
                    in1=bias_s[:].unsqueeze(1).broadcast_to([128, nblk, 128]),
                    op=Alu.add)
                nc.vector.tensor_scalar(out=s1[:], in0=hf, scalar1=0.0,
                                        scalar2=None, op0=Alu.min)
                nc.scalar.activation(s1[:], s1[:], Act.Exp)
                nc.vector.tensor_scalar(out=hf, in0=hf, scalar1=0.0,
                                        scalar2=None, op0=Alu.max)
                nc.vector.tensor_tensor(out=hf, in0=hf, in1=s1[:], op=Alu.add)
                nc.vector.tensor_scalar(out=hf, in0=hf, scalar1=-1.0,
                                        scalar2=None, op0=Alu.add)
                nc.vector.tensor_tensor(out=hf, in0=hf, in1=res[:], op=Alu.add)
                # LN over last dim
                red = sp.tile([128, nblk], F32, tag="red")
                nc.vector.tensor_reduce(out=red[:], in_=hf, op=Alu.add, axis=AxX)
                nc.vector.tensor_scalar(out=red[:], in0=red[:], scalar1=1.0 / 128,
                                        scalar2=None, op0=Alu.mult)
                nc.vector.tensor_tensor(
                    out=hf, in0=hf,
                    in1=red[:].unsqueeze(2).broadcast_to([128, nblk, 128]),
                    op=Alu.subtract)
                nc.vector.tensor_tensor(out=s1[:], in0=hf, in1=hf, op=Alu.mult)
                var = sp.tile([128, nblk], F32, tag="var")
                nc.vector.tensor_reduce(out=var[:], in_=s1[:], op=Alu.add, axis=AxX)
                nc.vector.tensor_scalar(out=var[:], in0=var[:],
                                        scalar1=1.0 / 128, scalar2=LN_EPS,
                                        op0=Alu.mult, op1=Alu.add)
                nc.scalar.activation(var[:], var[:], Act.Sqrt)
                nc.vector.reciprocal(var[:], var[:])
                nc.vector.tensor_tensor(
                    out=hf, in0=hf,
                    in1=var[:].unsqueeze(2).broadcast_to([128, nblk, 128]),
                    op=Alu.mult)
                nc.vector.tensor_tensor(
                    out=hf, in0=hf,
                    in1=gamma_s[:].unsqueeze(1).broadcast_to([128, nblk, 128]),
                    op=Alu.mult)
                nc.vector.tensor_tensor(
                    out=hf, in0=hf,
                    in1=beta_s[:].unsqueeze(1).broadcast_to([128, nblk, 128]),
                    op=Alu.add)
                if debug and hout_tap:
                    for bb in range(nblk):
                        tcp = wp.tile([128, 128], F32, tag="dbgcp")
                        nc.vector.tensor_copy(tcp[:], hout[:, bb, :])
                        nc.sync.dma_start(out=dbg_h1[bb * 128:(bb + 1) * 128, :], in_=tcp[:])

            h1 = bigf.tile([128, nblk, 128], F32, tag="bf")
            edge_layer(ag1, xr1, att1s, we1s, bias1s, res1, g1s, bn1s, h1, hout_tap=True)

            # ---- P2: transposes + layer-2 projections
            h1T = bigb.tile([128, nblk, 128], BF16, tag="bb")
            xr2 = bigb.tile([128, nblk, 128], BF16, tag="bb")
            for t in range(nblk):
                tp = psA.tile([128, 128], F32, tag="pa")
                nc.tensor.transpose(tp[:], h1[:, t, :], idents[:])
                nc.vector.tensor_copy(h1T[:, t, :], tp[:])
                ps = psA.tile([128, 256], F32, tag="pa")
                nc.tensor.matmul(ps[:], h1T[:, t, :], w2s[:], start=True, stop=True)
                pb = wp.tile([128, 256], F32, tag="projf2")
                nc.vector.tensor_tensor(out=pb[:], in0=ps[:], in1=b2s[:], op=Alu.add)
                xlb = wp.tile([128, 128], BF16, tag="projb")
                nc.vector.tensor_copy(xlb[:], pb[:, 0:128])
                nc.sync.dma_start(out=xl2d[t * 128:(t + 1) * 128, :], in_=xlb[:])
                nc.vector.tensor_copy(xr2[:, t, :], pb[:, 128:256])

            nc.gpsimd.collective_compute(
                "AllGather", Alu.bypass,
                replica_groups=[list(range(NCORES))],
                ins=[xl2d[:]], outs=[ag2[:]],
            )

            h2 = bigf.tile([128, nblk, 128], F32, tag="bf")
            edge_layer(ag2, xr2, att2s, we2s, bias2s, h1, g2s, bn2s, h2)

            # ---- output stage: out = LN(h2 @ Wout + bout)
            for t in range(nblk):
                tp = psA.tile([128, 128], F32, tag="pa")
                nc.tensor.transpose(tp[:], h2[:, t, :], idents[:])
                h2T = wp.tile([128, 128], BF16, tag="h2T")
                nc.vector.tensor_copy(h2T[:], tp[:])
                ps = psA.tile([128, 128], F32, tag="pa")
                nc.tensor.matmul(ps[:], h2T[:], wos[:], start=True, stop=True)
                ob = wp.tile([128, 128], F32, tag="ob")
                nc.vector.tensor_tensor(out=ob[:], in0=ps[:], in1=bos[:], op=Alu.add)
                red = sp.tile([128, 1], F32, tag="redo")
                nc.vector.tensor_reduce(out=red[:], in_=ob[:], op=Alu.add, axis=AxX)
                nc.vector.tensor_scalar(out=red[:], in0=red[:], scalar1=1.0 / 128,
                                        scalar2=None, op0=Alu.mult)
                nc.vector.tensor_scalar(out=ob[:], in0=ob[:], scalar1=red[:],
                                        scalar2=None, op0=Alu.subtract)
                sq = wp.tile([128, 128], F32, tag="sq")
                nc.vector.tensor_tensor(out=sq[:], in0=ob[:], in1=ob[:], op=Alu.mult)
                var = sp.tile([128, 1], F32, tag="varo")
                nc.vector.tensor_reduce(out=var[:], in_=sq[:], op=Alu.add, axis=AxX)
                nc.vector.tensor_scalar(out=var[:], in0=var[:], scalar1=1.0 / 128,
                                        scalar2=LN_EPS, op0=Alu.mult, op1=Alu.add)
                nc.scalar.activation(var[:], var[:], Act.Sqrt)
                nc.vector.reciprocal(var[:], var[:])
                nc.vector.tensor_scalar(out=ob[:], in0=ob[:], scalar1=var[:],
                                        scalar2=None, op0=Alu.mult)
                nc.vector.tensor_tensor(out=ob[:], in0=ob[:], in1=gos[:], op=Alu.mult)
                nc.vector.tensor_tensor(out=ob[:], in0=ob[:], in1=bo2s[:], op=Alu.add)
                nc.sync.dma_start(out=out_ext[t * 128:(t + 1) * 128, :], in_=ob[:])

    nc.compile()
    return nc


_CACHE = {}


def _install_ntff_shim():
    """Provide antenv.axon_hooks (missing on this image) so trace=True can
    drive NTFF profiling through the axon .so, and stub artifact upload."""
    import types, contextlib, ctypes
    import concourse.bass_utils as bu
    try:
        from antenv.axon_hooks import get_axon_ntff_profile_hook  # noqa: F401
    except ImportError:
        mod = types.ModuleType("antenv.axon_hooks")
        box = [None]
        mod.set_axon_ntff_profile_hook = lambda h: box.__setitem__(0, h)
        mod.get_axon_ntff_profile_hook = lambda: box[0]
        sys.modules["antenv.axon_hooks"] = mod
        import antenv
        antenv.axon_hooks = mod

        so_path = "/opt/axon/libaxon_pjrt.so"
        lib = ctypes.CDLL(so_path)
        if hasattr(lib, "axon_start_nrt_profile"):
            lib.axon_start_nrt_profile.argtypes = [
                ctypes.POINTER(ctypes.c_int64), ctypes.c_size_t]
            lib.axon_start_nrt_profile.restype = ctypes.c_int64
            lib.axon_stop_nrt_profile.argtypes = [ctypes.c_char_p]
            lib.axon_stop_nrt_profile.restype = ctypes.c_int64

            @contextlib.contextmanager
            def _hook(output_dir, device_ids):
                import jax
                jax.devices()
                if device_ids:
                    ids = (ctypes.c_int64 * len(device_ids))(*device_ids)
                    rc = lib.axon_start_nrt_profile(ids, len(device_ids))
                else:
                    rc = lib.axon_start_nrt_profile(None, 0)
                if rc != 0:
                    raise RuntimeError(f"axon_start_nrt_profile rc={rc}")
                try:
                    yield
                finally:
                    nf = lib.axon_stop_nrt_profile(str(output_dir).encode())
                    print(f"ntff profile: {nf} file(s) -> {output_dir}",
                          file=sys.stderr)

            mod.set_axon_ntff_profile_hook(_hook)

    bu.upload_artifacts = lambda tmpdir: f"local://{tmpdir}"



def prepare(inputs, nblk_per_core=49):
    x = np.asarray(inputs["x"], np.float32)
    edge_attr = np.asarray(inputs["edge_attr"], np.float32)
    edge_index = np.asarray(inputs["edge_index"])
    din = x.shape[1]
    ed = edge_attr.shape[1]
    per_core, meta = _prep(x, edge_attr, edge_index, nblk_per_core)
    cb, nblk = meta["cb"], meta["nblk"]

    Wl1, Wr1, Wres = inputs["Wl1"], inputs["Wr1"], inputs["Wres"]
    Wl2, Wr2 = inputs["Wl2"], inputs["Wr2"]
    cat = np.concatenate
    w1 = cat([Wl1, Wr1, Wres], 1).astype(np.float32)      # (din, 384)
    w2 = cat([Wl2, Wr2], 1).astype(np.float32)            # (128, 256)
    b1 = cat([np.ravel(inputs["bl1"]), np.ravel(inputs["br1"]),
              np.ravel(inputs["bres"])])
    b2 = cat([np.ravel(inputs["bl2"]), np.ravel(inputs["br2"])])
    iota = np.broadcast_to(np.arange(128, dtype=np.float32), (128, 128)).copy()

    common = dict(
        w1=_bf(w1), w2=_bf(w2), wo=_bf(inputs["Wout"]),
        we1=_bf(inputs["We1"]),
        we2=_bf(inputs["We2"]),
        b1r=_rep(b1), b2r=_rep(b2),
        bias1r=_rep(inputs["bias1"]), bias2r=_rep(inputs["bias2"]),
        bor=_rep(inputs["bout"]),
        att1r=_bf(_rep(np.ravel(inputs["att1"]))),
        att2r=_bf(_rep(np.ravel(inputs["att2"]))),
        g1r=_rep(inputs["g1"]), bn1r=_rep(inputs["bn1"]),
        g2r=_rep(inputs["g2"]), bn2r=_rep(inputs["bn2"]),
        gor=_rep(inputs["go"]), bo2r=_rep(inputs["bo"]),
        iotar=iota, iotac=np.arange(128, dtype=np.float32).reshape(128, 1),
        identm=np.eye(128, dtype=np.float32),
    )

    in_maps = []
    for c in range(NCORES):
        pc = per_core[c]
        xT = pc["xT"]
        m = dict(common)
        m.update(
            xT_a=_bf(xT[0:128].reshape(128, nblk, 128)),
            xT_b=_bf(xT[128:256].reshape(128, nblk, 128)),
            xT_c=_bf(xT[256:din].reshape(din - 256, nblk, 128)),
            eidx=pc["eidx"], epar=pc["epar"], edrel=pc["edrel"],
            edrr=pc["edrr"], eaT=_bf(pc["eaT"]),
        )
        in_maps.append(m)

    return in_maps, meta


def kernel(_want_trace=False, **inputs):
    if _want_trace:
        _install_ntff_shim()
    in_maps, meta = prepare(inputs)
    cb, nblk = meta["cb"], meta["nblk"]
    din, ed = meta["din"], meta["ed"]
    key = (cb, din, ed, nblk)
    if key not in _CACHE:
        _CACHE[key] = build_graph(cb, din, ed, nblk)
    nc = _CACHE[key]
    res = run_bass_kernel_spmd(nc, in_maps, list(range(NCORES)),
                               trace=_want_trace)
    outs = np.concatenate([res.results[c]["out"] for c in range(NCORES)], 0)
    full = outs[meta["new_of_old"]].astype(np.float32)
    if _want_trace:
        return full, res
    return full


# revision 10
# speedup vs baseline: 1.4476x; 1.4476x over previous
"""Distributed Trainium2 (8 NeuronCores) kernel for a 2-layer GATv2 GNN.

Self-contained: takes full inputs, shards by destination node across 8 cores,
runs one SPMD Bass kernel (compiled at call time, cached per process), and
returns the full (50000, 128) output.

Sharding: nodes are relabeled so that in-degrees are balanced across 392
dst-blocks of 128 nodes (greedy LPT); each core owns 49 blocks. Edges live on
the core owning their destination; segment softmax and scatter-add are
device-local. Source features cross cores via an AllGather of the per-layer
source projection (bf16), then per-edge rows are fetched with dma_gather from
the gathered table (2 nodes packed per 512B element so indices fit in int16).
"""

import os, sys, math
sys.path.insert(0, "/opt/trn_rl_repo")
os.environ.setdefault("JAX_PLATFORMS", "")

import numpy as np
import ml_dtypes

import concourse.bacc as bacc
import concourse.bass as bass
import concourse.mybir as mybir
import concourse.tile as tile
from concourse.bass_utils import run_bass_kernel_spmd

F32 = mybir.dt.float32
BF16 = mybir.dt.bfloat16
I16 = mybir.dt.int16
U8 = mybir.dt.uint8
Alu = mybir.AluOpType
Act = mybir.ActivationFunctionType
AxX = mybir.AxisListType.X

NCORES = 8
BLK = 128
NEG_SLOPE = 0.2
LN_EPS = 1e-5
H, C = 4, 32
HID = H * C  # 128


# ---------------------------------------------------------------- host prep

def _balance_blocks(dst, n_nodes, n_blocks):
    """Greedy LPT: assign nodes to n_blocks blocks of <=128 nodes balancing
    in-degree sums. Returns block_of_node (int32)."""
    import heapq
    deg = np.bincount(dst, minlength=n_nodes).astype(np.int64)
    order = np.argsort(-deg, kind="stable")
    heap = [(0, b) for b in range(n_blocks)]
    heapq.heapify(heap)
    counts = np.zeros(n_blocks, np.int32)
    block_of = np.empty(n_nodes, np.int32)
    stash = []
    for nid in order:
        d = int(deg[nid])
        while True:
            s, b = heapq.heappop(heap)
            if counts[b] < BLK:
                break
            # full block: drop it permanently
        block_of[nid] = b
        counts[b] += 1
        if counts[b] < BLK:
            heapq.heappush(heap, (s + d, b))
        else:
            stash.append((s + d, b))
    return block_of, deg


def _wrap_idx(idx1d):
    """int16 (E,) -> (128, E//16) wrapped-16 + replicated layout."""
    e = idx1d.shape[0]
    assert e % 16 == 0
    w = idx1d.reshape(e // 16, 16).T  # (16, S)
    return np.tile(w, (8, 1)).copy()


def _prep(x, edge_attr, edge_index, nblk_per_core=49):
    n, din = x.shape
    e = edge_index.shape[1]
    n_blocks = NCORES * nblk_per_core  # 392
    npad = n_blocks * BLK              # 50176
    npc = nblk_per_core * BLK          # 6272

    src, dst = edge_index[0].astype(np.int64), edge_index[1].astype(np.int64)
    block_of, deg = _balance_blocks(dst, n, n_blocks)

    # new node ids: block b slots
    order = np.argsort(block_of, kind="stable")
    new_id = np.empty(npad, np.int64)
    # slot within block
    counts = np.bincount(block_of, minlength=n_blocks)
    starts = np.zeros(n_blocks + 1, np.int64)
    np.cumsum(counts, out=starts[1:])
    slot = np.empty(n, np.int64)
    slot[order] = np.arange(n) - starts[block_of[order]]
    new_of_old = block_of.astype(np.int64) * BLK + slot  # old -> new
    # per-block edge counts and capacity
    eb = np.bincount(block_of[dst], minlength=n_blocks)
    cb = int(math.ceil(eb.max() / BLK))
    ecap = cb * BLK

    new_src = new_of_old[src]
    new_dst = new_of_old[dst]
    dblk = new_dst // BLK

    # order edges by destination block
    eorder = np.argsort(dblk, kind="stable")
    s_src = new_src[eorder]
    s_dst = new_dst[eorder]
    s_att = edge_attr[eorder]
    s_blk = dblk[eorder]

    # slot edges into (n_blocks, ecap) with padding
    eoff = np.zeros(n_blocks + 1, np.int64)
    np.cumsum(eb, out=eoff[1:])
    in_blk_pos = np.arange(e) - eoff[s_blk]
    flat_pos = s_blk * ecap + in_blk_pos

    g_srcpk = np.zeros(n_blocks * ecap, np.int16)
    g_par = np.zeros(n_blocks * ecap, np.uint8)
    g_drel = np.full(n_blocks * ecap, -1.0, np.float32)
    g_attT = np.zeros((n_blocks * ecap, edge_attr.shape[1]), np.float32)
    g_srcpk[flat_pos] = (s_src // 2).astype(np.int16)
    g_par[flat_pos] = (s_src % 2).astype(np.uint8)
    g_drel[flat_pos] = (s_dst % BLK).astype(np.float32)
    g_attT[flat_pos] = s_att

    g_srcpk = g_srcpk.reshape(n_blocks, ecap)
    g_par = g_par.reshape(n_blocks, ecap)
    g_drel = g_drel.reshape(n_blocks, ecap)
    g_attT = g_attT.reshape(n_blocks, ecap, -1)

    # x in new order, padded, transposed per core
    x_new = np.zeros((npad, din), np.float32)
    x_new[new_of_old] = x

    per_core = []
    for c in range(NCORES):
        b0, b1 = c * nblk_per_core, (c + 1) * nblk_per_core
        srcpk = g_srcpk[b0:b1]
        # wrapped idx layout per block: (49, 128, cb*8)
        eidx = np.stack([_wrap_idx(srcpk[i]) for i in range(nblk_per_core)])
        # (49, 128, cb) partition-major per chunk: elem j of chunk k sits at
        # partition j, free k  -> arr[b, p, k] = val[b, k*128 + p]
        def pmaj(a, dt):
            return np.ascontiguousarray(
                a.reshape(nblk_per_core, cb, BLK).transpose(0, 2, 1)).astype(dt)
        epar = pmaj(g_par[b0:b1], np.uint8)
        edrel = pmaj(g_drel[b0:b1], np.float32)  # (49, 128, cb)
        # prebuilt one-hots (bf16 on device): oh[b, p, k, d], ohT[b, d, e]
        dvals = np.arange(BLK, dtype=np.float32)
        eoh = (edrel[..., None] == dvals).astype(ml_dtypes.bfloat16)
        drf = g_drel[b0:b1]  # (49, ecap) flat edge order
        eohT = (dvals[None, :, None] == drf[:, None, :]).astype(ml_dtypes.bfloat16)
        eaT = np.ascontiguousarray(
            g_attT[b0:b1].transpose(0, 2, 1)).astype(np.float32)  # (49,16,ecap)
        xT = np.ascontiguousarray(x_new[c * npc:(c + 1) * npc].T)  # (din, 6272)
        per_core.append(dict(eidx=eidx, epar=epar, eoh=eoh, eohT=eohT,
                             eaT=eaT, xT=xT))
    meta = dict(cb=cb, npad=npad, npc=npc, nblk=nblk_per_core,
                new_of_old=new_of_old, din=din, ed=edge_attr.shape[1])
    return per_core, meta


# ---------------------------------------------------------------- builder

def _rep(v, rows=128):
    v = np.asarray(v, np.float32).ravel()
    return np.broadcast_to(v, (rows, v.shape[0])).copy()


def _bf(a):
    return np.asarray(a).astype(ml_dtypes.bfloat16)


def build_graph(cb, din, ed, nblk=49, debug=False):
    npc = nblk * BLK
    ecap = cb * BLK
    npad = NCORES * npc
    nc = bacc.Bacc(None, target_bir_lowering=False)

    dp = nc.declare_dram_parameter
    # per-core data
    xT_a = dp("xT_a", [128, nblk, 128], BF16, isOutput=False)
    xT_b = dp("xT_b", [128, nblk, 128], BF16, isOutput=False)
    xT_c = dp("xT_c", [din - 256, nblk, 128], BF16, isOutput=False)
    eidx = dp("eidx", [nblk, 128, ecap // 16], I16, isOutput=False)
    epar = dp("epar", [nblk, 128, cb], U8, isOutput=False)
    eoh = dp("eoh", [nblk, 128, cb, 128], BF16, isOutput=False)
    eohT = dp("eohT", [nblk, 128, ecap], BF16, isOutput=False)
    eaT = dp("eaT", [nblk, ed, ecap], BF16, isOutput=False)
    # replicated weights
    w1 = dp("w1", [din, 384], BF16, isOutput=False)      # [Wl1|Wr1|Wres]
    w2 = dp("w2", [HID, 256], BF16, isOutput=False)      # [Wl2|Wr2]
    wo = dp("wo", [HID, 128], BF16, isOutput=False)
    we1 = dp("we1", [ed, 128], BF16, isOutput=False)
    we2 = dp("we2", [ed, 128], BF16, isOutput=False)
    b1r = dp("b1r", [128, 384], F32, isOutput=False)     # [bl1|br1|bres] rep
    b2r = dp("b2r", [128, 256], F32, isOutput=False)
    bias1r = dp("bias1r", [128, 128], F32, isOutput=False)
    bias2r = dp("bias2r", [128, 128], F32, isOutput=False)
    bor = dp("bor", [128, 128], F32, isOutput=False)
    att1r = dp("att1r", [128, 128], BF16, isOutput=False)
    att2r = dp("att2r", [128, 128], BF16, isOutput=False)
    g1r = dp("g1r", [128, 128], F32, isOutput=False)
    bn1r = dp("bn1r", [128, 128], F32, isOutput=False)
    g2r = dp("g2r", [128, 128], F32, isOutput=False)
    bn2r = dp("bn2r", [128, 128], F32, isOutput=False)
    gor = dp("gor", [128, 128], F32, isOutput=False)
    bo2r = dp("bo2r", [128, 128], F32, isOutput=False)
    identm = dp("identm", [128, 128], F32, isOutput=False)
    out_ext = dp("out", [npc, 128], F32, isOutput=True)

    xl1d = nc.dram_tensor("xl1d", [npc, 128], BF16)
    if debug:
        dbg_hraw = nc.declare_dram_parameter("dbg_hraw", [npc, 128], F32, isOutput=True)
        dbg_h1 = nc.declare_dram_parameter("dbg_h1", [npc, 128], F32, isOutput=True)
        dbg_xls = nc.declare_dram_parameter("dbg_xls", [128, cb, 128], F32, isOutput=True)
        dbg_sc = nc.declare_dram_parameter("dbg_sc", [128, cb, 4], F32, isOutput=True)
    xl2d = nc.dram_tensor("xl2d", [npc, 128], BF16)
    ag1 = nc.dram_tensor("ag1", [npad, 128], BF16, addr_space="Shared")
    ag2 = nc.dram_tensor("ag2", [npad, 128], BF16, addr_space="Shared")

    with tile.TileContext(nc) as tc:
        with (
            tc.tile_pool(name="const", bufs=1) as cpool,
            tc.tile_pool(name="bigb", bufs=3) as bigb,
            tc.tile_pool(name="bigf", bufs=3) as bigf,
            tc.tile_pool(name="bigc", bufs=1) as bigc,
            tc.tile_pool(name="work", bufs=2) as wp,
            tc.tile_pool(name="small", bufs=3) as sp,
            tc.tile_pool(name="psA", bufs=2, space="PSUM") as psA,
            tc.tile_pool(name="psB", bufs=2, space="PSUM") as psB,
            tc.tile_pool(name="psC", bufs=2, space="PSUM") as psC,
        ):
            # ---- constants in SBUF
            def cload(ap, shape, dt, tag):
                t = cpool.tile(shape, dt, tag=tag)
                if hasattr(ap, 'shape') and not isinstance(ap, bass.AP):
                    ap = ap[:]
                nc.sync.dma_start(out=t[:], in_=ap)
                return t

            w1a = cload(w1[0:128, :], [128, 384], BF16, "w1a")
            w1b = cload(w1[128:256, :], [128, 384], BF16, "w1b")
            w1c = cload(w1[256:din, :], [din - 256, 384], BF16, "w1c")
            w2s = cload(w2[:], [HID, 256], BF16, "w2s")
            wos = cload(wo[:], [HID, 128], BF16, "wos")
            we1s = cload(we1[:], [ed, 128], BF16, "we1s")
            we2s = cload(we2[:], [ed, 128], BF16, "we2s")
            b1s = cload(b1r[:], [128, 384], F32, "b1s")
            b2s = cload(b2r[:], [128, 256], F32, "b2s")
            bias1s = cload(bias1r[:], [128, 128], F32, "bias1s")
            bias2s = cload(bias2r[:], [128, 128], F32, "bias2s")
            bos = cload(bor[:], [128, 128], F32, "bos")
            att1s = cload(att1r[:], [128, 128], BF16, "att1s")
            att2s = cload(att2r[:], [128, 128], BF16, "att2s")
            g1s = cload(g1r[:], [128, 128], F32, "g1s")
            bn1s = cload(bn1r[:], [128, 128], F32, "bn1s")
            g2s = cload(g2r[:], [128, 128], F32, "g2s")
            bn2s = cload(bn2r[:], [128, 128], F32, "bn2s")
            gos = cload(gor[:], [128, 128], F32, "gos")
            bo2s = cload(bo2r[:], [128, 128], F32, "bo2s")
            idents = cload(identm[:], [128, 128], F32, "idents")

            # ---- persistent big tiles
            xa = bigb.tile([128, nblk, 128], BF16, tag="bb")
            xb = bigb.tile([128, nblk, 128], BF16, tag="bb")
            xc_ = bigc.tile([din - 256, nblk, 128], BF16, tag="bbc")
            nc.sync.dma_start(out=xa[:], in_=xT_a[:])
            nc.sync.dma_start(out=xb[:], in_=xT_b[:])
            nc.sync.dma_start(out=xc_[:], in_=xT_c[:])

            res1 = bigf.tile([128, nblk, 128], F32, tag="bf")
            xr1 = bigb.tile([128, nblk, 128], BF16, tag="bb")

            # ---- P1: layer-1 projections per node tile
            for t in range(nblk):
                ps = psA.tile([128, 384], F32, tag="pa")
                nc.tensor.matmul(ps[:], xa[:, t, :], w1a[:], start=True, stop=False)
                nc.tensor.matmul(ps[:], xb[:, t, :], w1b[:], start=False, stop=False)
                nc.tensor.matmul(ps[:], xc_[:, t, :], w1c[:], start=False, stop=True)
                pb = wp.tile([128, 384], F32, tag="projf")
                nc.vector.tensor_tensor(out=pb[:], in0=ps[:], in1=b1s[:], op=Alu.add)
                xlb = wp.tile([128, 128], BF16, tag="projb")
                nc.scalar.copy(xlb[:], pb[:, 0:128])
                nc.sync.dma_start(out=xl1d[t * 128:(t + 1) * 128, :], in_=xlb[:])
                nc.scalar.copy(xr1[:, t, :], pb[:, 128:256])
                nc.scalar.copy(res1[:, t, :], pb[:, 256:384])

            nc.gpsimd.collective_compute(
                "AllGather", Alu.bypass,
                replica_groups=[list(range(NCORES))],
                ins=[xl1d[:]], outs=[ag1[:]],
            )

            # ---- edge stage (shared for both layers)
            def edge_layer(ag, xr, att_s, we_s, bias_s, res, gamma_s, beta_s,
                           hout, hout_tap=False):
                agp = ag[:].rearrange("(n two) f -> n (two f)", two=2)
                for b in range(nblk):
                    idxt = wp.tile([128, ecap // 16], I16, tag="idxt")
                    nc.sync.dma_start(out=idxt[:], in_=eidx[b])
                    part = wp.tile([128, cb], U8, tag="part")
                    nc.sync.dma_start(out=part[:], in_=epar[b])
                    oh = wp.tile([128, cb, 128], BF16, tag="oh")
                    nc.sync.dma_start(out=oh[:], in_=eoh[b])
                    ohT = wp.tile([128, ecap], BF16, tag="ohT")
                    nc.sync.dma_start(out=ohT[:], in_=eohT[b])
                    eat = wp.tile([ed, ecap], BF16, tag="eat")
                    nc.sync.dma_start(out=eat[:], in_=eaT[b])

                    gth = wp.tile([128, cb, 256], BF16, tag="gth")
                    nc.gpsimd.dma_gather(
                        out_ap=gth[:], in_ap=agp, idxs_ap=idxt[:],
                        num_idxs=ecap, num_idxs_reg=ecap, elem_size=256,
                        single_packet=False)

                    xls = wp.tile([128, cb, 128], BF16, tag="xls")
                    nc.scalar.copy(xls[:], gth[:, :, 0:128])
                    for k in range(cb):
                        nc.vector.copy_predicated(
                            xls[:, k, :],
                            part[:, k:k + 1].broadcast_to([128, 128]),
                            gth[:, k, 128:256])


                    # q = onehotT^T @ xr_blk + eaT^T @ We ; m = xls + q
                    mt = wp.tile([128, cb, 128], BF16, tag="mt")
                    for k in range(cb):
                        qps = psB.tile([128, 128], F32, tag="pb")
                        nc.tensor.matmul(qps[:], ohT[:, k * 128:(k + 1) * 128],
                                         xr[:, b, :], start=True, stop=False)
                        nc.tensor.matmul(qps[:], eat[:, k * 128:(k + 1) * 128],
                                         we_s[:], start=False, stop=True)
                        nc.vector.tensor_tensor(out=mt[:, k, :], in0=xls[:, k, :],
                                                in1=qps[:], op=Alu.add)

                    if debug and hout_tap and b == 0:
                        xcp = wp.tile([128, cb, 128], F32, tag="dbgx")
                        nc.vector.tensor_copy(xcp[:], mt[:])
                        nc.sync.dma_start(out=dbg_xls[:], in_=xcp[:])
                    # leaky relu: g = max(m, 0.2 m)
                    m2 = wp.tile([128, cb, 128], BF16, tag="m2")
                    nc.scalar.mul(m2[:], mt[:], NEG_SLOPE)
                    nc.vector.tensor_tensor(out=mt[:], in0=mt[:], in1=m2[:],
                                            op=Alu.max)
                    # scores: reuse m2 as g*att
                    gs = m2
                    nc.vector.tensor_tensor(
                        out=gs[:], in0=mt[:],
                        in1=att_s[:].unsqueeze(1).broadcast_to([128, cb, 128]),
                        op=Alu.mult)
                    sc = sp.tile([128, cb, H], F32, tag="sc")
                    nc.vector.tensor_reduce(
                        out=sc[:], in_=gs[:].rearrange("p c (h j) -> p c h j", h=H),
                        op=Alu.add, axis=AxX)
                    if debug and hout_tap and b == 0:
                        nc.sync.dma_start(out=dbg_sc[:], in_=sc[:])
                    ex = sp.tile([128, cb, H], F32, tag="ex")
                    nc.scalar.activation(ex[:], sc[:], Act.Exp)

                    rhsb = wp.tile([128, cb, 132], BF16, tag="rhsb")
                    nc.vector.tensor_tensor(
                        out=rhsb[:, :, 0:128].rearrange("p c (h j) -> p c h j", h=H),
                        in0=xls[:].rearrange("p c (h j) -> p c h j", h=H),
                        in1=ex[:].unsqueeze(3).broadcast_to([128, cb, H, C]),
                        op=Alu.mult)
                    nc.scalar.copy(rhsb[:, :, 128:132], ex[:])

                    scps = psC.tile([128, 132], F32, tag="pc")
                    for k in range(cb):
                        nc.tensor.matmul(scps[:], oh[:, k, :], rhsb[:, k, :],
                                         start=(k == 0), stop=(k == cb - 1))

                    st = sp.tile([128, H], F32, tag="st")
                    nc.vector.tensor_scalar(out=st[:], in0=scps[:, 128:132],
                                            scalar1=1e-16, scalar2=None,
                                            op0=Alu.add)
                    srec = sp.tile([128, H], F32, tag="srec")
                    nc.vector.reciprocal(srec[:], st[:])
                    nc.vector.tensor_tensor(
                        out=hout[:, b, :].rearrange("p (h j) -> p h j", h=H),
                        in0=scps[:, 0:128].rearrange("p (h j) -> p h j", h=H),
                        in1=srec[:].unsqueeze(2).broadcast_to([128, H, C]),
                        op=Alu.mult)

                if debug and hout_tap:
                    for bb in range(nblk):
                        tcp = wp.tile([128, 128], F32, tag="dbgcp")
                        nc.vector.tensor_copy(tcp[:], hout[:, bb, :])
                        nc.sync.dma_start(out=dbg_hraw[bb * 128:(bb + 1) * 128, :], in_=tcp[:])
                # batched: h = LN(elu(h + bias) + res)
                s1 = bigf.tile([128, nblk, 128], F32, tag="bf")
                hf = hout[:]
                nc.vector.tensor_tensor(
                    out=hf, in0=hf,
                    in1=bias_s[:].unsqueeze(1).broadcast_to([128, nblk, 128]),
                    op=Alu.add)
                nc.vector.tensor_scalar(out=s1[:], in0=hf, scalar1=0.0,
                                        scalar2=None, op0=Alu.min)
                nc.scalar.activation(s1[:], s1[:], Act.Exp)
                nc.vector.tensor_scalar(out=hf, in0=hf, scalar1=0.0,
                                        scalar2=None, op0=Alu.max)
                nc.vector.tensor_tensor(out=hf, in0=hf, in1=s1[:], op=Alu.add)
                nc.vector.tensor_scalar(out=hf, in0=hf, scalar1=-1.0,
                                        scalar2=None, op0=Alu.add)
                nc.vector.tensor_tensor(out=hf, in0=hf, in1=res[:], op=Alu.add)
                # LN over last dim
                red = sp.tile([128, nblk], F32, tag="red")
                nc.vector.tensor_reduce(out=red[:], in_=hf, op=Alu.add, axis=AxX)
                nc.vector.tensor_scalar(out=red[:], in0=red[:], scalar1=1.0 / 128,
                                        scalar2=None, op0=Alu.mult)
                nc.vector.tensor_tensor(
                    out=hf, in0=hf,
                    in1=red[:].unsqueeze(2).broadcast_to([128, nblk, 128]),
                    op=Alu.subtract)
                nc.vector.tensor_tensor(out=s1[:], in0=hf, in1=hf, op=Alu.mult)
                var = sp.tile([128, nblk], F32, tag="var")
                nc.vector.tensor_reduce(out=var[:], in_=s1[:], op=Alu.add, axis=AxX)
                nc.vector.tensor_scalar(out=var[:], in0=var[:],
                                        scalar1=1.0 / 128, scalar2=LN_EPS,
                                        op0=Alu.mult, op1=Alu.add)
                nc.scalar.activation(var[:], var[:], Act.Sqrt)
                nc.vector.reciprocal(var[:], var[:])
                nc.vector.tensor_tensor(
                    out=hf, in0=hf,
                    in1=var[:].unsqueeze(2).broadcast_to([128, nblk, 128]),
                    op=Alu.mult)
                nc.vector.tensor_tensor(
                    out=hf, in0=hf,
                    in1=gamma_s[:].unsqueeze(1).broadcast_to([128, nblk, 128]),
                    op=Alu.mult)
                nc.vector.tensor_tensor(
                    out=hf, in0=hf,
                    in1=beta_s[:].unsqueeze(1).broadcast_to([128, nblk, 128]),
                    op=Alu.add)
                if debug and hout_tap:
                    for bb in range(nblk):
                        tcp = wp.tile([128, 128], F32, tag="dbgcp")
                        nc.vector.tensor_copy(tcp[:], hout[:, bb, :])
                        nc.sync.dma_start(out=dbg_h1[bb * 128:(bb + 1) * 128, :], in_=tcp[:])

            h1 = bigf.tile([128, nblk, 128], F32, tag="bf")
            edge_layer(ag1, xr1, att1s, we1s, bias1s, res1, g1s, bn1s, h1, hout_tap=True)

            # ---- P2: transposes + layer-2 projections
            h1T = bigb.tile([128, nblk, 128], BF16, tag="bb")
            xr2 = bigb.tile([128, nblk, 128], BF16, tag="bb")
            for t in range(nblk):
                tp = psA.tile([128, 128], F32, tag="pa")
                nc.tensor.transpose(tp[:], h1[:, t, :], idents[:])
                nc.scalar.copy(h1T[:, t, :], tp[:])
                ps = psA.tile([128, 256], F32, tag="pa")
                nc.tensor.matmul(ps[:], h1T[:, t, :], w2s[:], start=True, stop=True)
                pb = wp.tile([128, 256], F32, tag="projf2")
                nc.vector.tensor_tensor(out=pb[:], in0=ps[:], in1=b2s[:], op=Alu.add)
                xlb = wp.tile([128, 128], BF16, tag="projb")
                nc.scalar.copy(xlb[:], pb[:, 0:128])
                nc.sync.dma_start(out=xl2d[t * 128:(t + 1) * 128, :], in_=xlb[:])
                nc.scalar.copy(xr2[:, t, :], pb[:, 128:256])

            nc.gpsimd.collective_compute(
                "AllGather", Alu.bypass,
                replica_groups=[list(range(NCORES))],
                ins=[xl2d[:]], outs=[ag2[:]],
            )

            h2 = bigf.tile([128, nblk, 128], F32, tag="bf")
            edge_layer(ag2, xr2, att2s, we2s, bias2s, h1, g2s, bn2s, h2)

            # ---- output stage: out = LN(h2 @ Wout + bout)
            for t in range(nblk):
                tp = psA.tile([128, 128], F32, tag="pa")
                nc.tensor.transpose(tp[:], h2[:, t, :], idents[:])
                h2T = wp.tile([128, 128], BF16, tag="h2T")
                nc.scalar.copy(h2T[:], tp[:])
                ps = psA.tile([128, 128], F32, tag="pa")
                nc.tensor.matmul(ps[:], h2T[:], wos[:], start=True, stop=True)
                ob = wp.tile([128, 128], F32, tag="ob")
                nc.vector.tensor_tensor(out=ob[:], in0=ps[:], in1=bos[:], op=Alu.add)
                red = sp.tile([128, 1], F32, tag="redo")
                nc.vector.tensor_reduce(out=red[:], in_=ob[:], op=Alu.add, axis=AxX)
                nc.vector.tensor_scalar(out=red[:], in0=red[:], scalar1=1.0 / 128,
                                        scalar2=None, op0=Alu.mult)
                nc.vector.tensor_scalar(out=ob[:], in0=ob[:], scalar1=red[:],
                                        scalar2=None, op0=Alu.subtract)
                sq = wp.tile([128, 128], F32, tag="sq")
                nc.vector.tensor_tensor(out=sq[:], in0=ob[:], in1=ob[:], op=Alu.mult)
                var = sp.tile([128, 1], F32, tag="varo")
                nc.vector.tensor_reduce(out=var[:], in_=sq[:], op=Alu.add, axis=AxX)
                nc.vector.tensor_scalar(out=var[:], in0=var[:], scalar1=1.0 / 128,
                                        scalar2=LN_EPS, op0=Alu.mult, op1=Alu.add)
                nc.scalar.activation(var[:], var[:], Act.Sqrt)
                nc.vector.reciprocal(var[:], var[:])
                nc.vector.tensor_scalar(out=ob[:], in0=ob[:], scalar1=var[:],
                                        scalar2=None, op0=Alu.mult)
                nc.vector.tensor_tensor(out=ob[:], in0=ob[:], in1=gos[:], op=Alu.mult)
                nc.vector.tensor_tensor(out=ob[:], in0=ob[:], in1=bo2s[:], op=Alu.add)
                nc.sync.dma_start(out=out_ext[t * 128:(t + 1) * 128, :], in_=ob[:])

    nc.compile()
    return nc


_CACHE = {}


def _install_ntff_shim():
    """Provide antenv.axon_hooks (missing on this image) so trace=True can
    drive NTFF profiling through the axon .so, and stub artifact upload."""
    import types, contextlib, ctypes
    import concourse.bass_utils as bu
    try:
        from antenv.axon_hooks import get_axon_ntff_profile_hook  # noqa: F401
    except ImportError:
        mod = types.ModuleType("antenv.axon_hooks")
        box = [None]
        mod.set_axon_ntff_profile_hook = lambda h: box.__setitem__(0, h)
        mod.get_axon_ntff_profile_hook = lambda: box[0]
        sys.modules["antenv.axon_hooks"] = mod
        import antenv
        antenv.axon_hooks = mod

        so_path = "/opt/axon/libaxon_pjrt.so"
        lib = ctypes.CDLL(so_path)
        if hasattr(lib, "axon_start_nrt_profile"):
            lib.axon_start_nrt_profile.argtypes = [
                ctypes.POINTER(ctypes.c_int64), ctypes.c_size_t]
            lib.axon_start_nrt_profile.restype = ctypes.c_int64
            lib.axon_stop_nrt_profile.argtypes = [ctypes.c_char_p]
            lib.axon_stop_nrt_profile.restype = ctypes.c_int64

            @contextlib.contextmanager
            def _hook(output_dir, device_ids):
                import jax
                jax.devices()
                if device_ids:
                    ids = (ctypes.c_int64 * len(device_ids))(*device_ids)
                    rc = lib.axon_start_nrt_profile(ids, len(device_ids))
                else:
                    rc = lib.axon_start_nrt_profile(None, 0)
                if rc != 0:
                    raise RuntimeError(f"axon_start_nrt_profile rc={rc}")
                try:
                    yield
                finally:
                    nf = lib.axon_stop_nrt_profile(str(output_dir).encode())
                    print(f"ntff profile: {nf} file(s) -> {output_dir}",
                          file=sys.stderr)

            mod.set_axon_ntff_profile_hook(_hook)

    bu.upload_artifacts = lambda tmpdir: f"local://{tmpdir}"



def prepare(inputs, nblk_per_core=49):
    x = np.asarray(inputs["x"], np.float32)
    edge_attr = np.asarray(inputs["edge_attr"], np.float32)
    edge_index = np.asarray(inputs["edge_index"])
    din = x.shape[1]
    ed = edge_attr.shape[1]
    per_core, meta = _prep(x, edge_attr, edge_index, nblk_per_core)
    cb, nblk = meta["cb"], meta["nblk"]

    Wl1, Wr1, Wres = inputs["Wl1"], inputs["Wr1"], inputs["Wres"]
    Wl2, Wr2 = inputs["Wl2"], inputs["Wr2"]
    cat = np.concatenate
    w1 = cat([Wl1, Wr1, Wres], 1).astype(np.float32)      # (din, 384)
    w2 = cat([Wl2, Wr2], 1).astype(np.float32)            # (128, 256)
    b1 = cat([np.ravel(inputs["bl1"]), np.ravel(inputs["br1"]),
              np.ravel(inputs["bres"])])
    b2 = cat([np.ravel(inputs["bl2"]), np.ravel(inputs["br2"])])
    iota = np.broadcast_to(np.arange(128, dtype=np.float32), (128, 128)).copy()

    common = dict(
        w1=_bf(w1), w2=_bf(w2), wo=_bf(inputs["Wout"]),
        we1=_bf(inputs["We1"]),
        we2=_bf(inputs["We2"]),
        b1r=_rep(b1), b2r=_rep(b2),
        bias1r=_rep(inputs["bias1"]), bias2r=_rep(inputs["bias2"]),
        bor=_rep(inputs["bout"]),
        att1r=_bf(_rep(np.ravel(inputs["att1"]))),
        att2r=_bf(_rep(np.ravel(inputs["att2"]))),
        g1r=_rep(inputs["g1"]), bn1r=_rep(inputs["bn1"]),
        g2r=_rep(inputs["g2"]), bn2r=_rep(inputs["bn2"]),
        gor=_rep(inputs["go"]), bo2r=_rep(inputs["bo"]),
        identm=np.eye(128, dtype=np.float32),
    )

    in_maps = []
    for c in range(NCORES):
        pc = per_core[c]
        xT = pc["xT"]
        m = dict(common)
        m.update(
            xT_a=_bf(xT[0:128].reshape(128, nblk, 128)),
            xT_b=_bf(xT[128:256].reshape(128, nblk, 128)),
            xT_c=_bf(xT[256:din].reshape(din - 256, nblk, 128)),
            eidx=pc["eidx"], epar=pc["epar"], eoh=pc["eoh"],
            eohT=pc["eohT"], eaT=_bf(pc["eaT"]),
        )
        in_maps.append(m)

    return in_maps, meta


def kernel(_want_trace=False, **inputs):
    if _want_trace:
        _install_ntff_shim()
    in_maps, meta = prepare(inputs)
    cb, nblk = meta["cb"], meta["nblk"]
    din, ed = meta["din"], meta["ed"]
    key = (cb, din, ed, nblk)
    if key not in _CACHE:
        _CACHE[key] = build_graph(cb, din, ed, nblk)
    nc = _CACHE[key]
    res = run_bass_kernel_spmd(nc, in_maps, list(range(NCORES)),
                               trace=_want_trace)
    outs = np.concatenate([res.results[c]["out"] for c in range(NCORES)], 0)
    full = outs[meta["new_of_old"]].astype(np.float32)
    if _want_trace:
        return full, res
    return full
